# revision 24
# baseline (speedup 1.0000x reference)
"""AttentionPooling Trainium2 Bass kernel (v2).

Full inputs in, full outputs out. Data-parallel over batch across 8 cores
(2 batches per core). Host folds the query/K projections into one small
[D, H] matrix qkt (scores[b,s,h] = tokens[b,s,:] @ qkt); V/O projections
are deferred until after the sequence reduction.

v2 reads tokens from HBM exactly once, in bf16 (24 MiB/core vs 72 in v1):

  per 128-token subtile:
    - 12 PE transposes of the bf16 token tile -> tt[d, s]  (bf16 PSUM),
      copied to SBUF alternately by DVE / scalar engine
    - scoresT[s, h]: 12 accumulating matmuls, lhsT = tt_j, rhs = qkt_j
    - exp on the scalar engine; the key-padding mask rides the per-
      partition activation bias (tokens are partitions here)
    - pooledT[d, h] accumulates in PSUM via lhsT = token tile (stationary),
      rhs = exp(scoresT); the softmax normalizer Z is one extra ones-column
      matmul into a [1, H] PSUM accumulator

  The 1/Z normalization is applied after the V-projection, where Z is a
  per-partition [B_LOC, 1] scalar per head block. Weights are bf16 and
  stream on the gpsimd SWDGE ring, overlapping the token stream.

Optional trmode "peN": N of the 12 d-tiles per subtile come pre-transposed
from HBM (host-prepared layout, contiguous DMA) instead of PE transposes,
trading DMA bytes for PE cycles.
"""

import numpy as np

B, S, D, H = 16, 4096, 1536, 8
HD = D // H                     # 192
N_CORES = 8
B_LOC = B // N_CORES            # 2 batches per core
NJ = D // 128                   # 12 d-tiles
TS = S // 128                   # 32 subtiles per batch
CT = 512                        # tokens per streamed chunk
EPS = 1e-6

_CACHE = {}


def _build_nc(reps=1, ct=CT, chunk_bufs=3, tt_bufs=3, masked=False,
              biased=False, trmode="pe0", copies="mix", ablate="none",
              grp=6, dbg=False):
    import concourse.bacc as bacc
    import concourse.tile as tile
    from concourse import mybir
    from concourse.masks import make_identity

    ndmat = int(trmode[2:]) if trmode.startswith("pe") else 0

    f32 = mybir.dt.float32
    bf16 = mybir.dt.bfloat16
    Exp = mybir.ActivationFunctionType.Exp
    Sqrt = mybir.ActivationFunctionType.Sqrt

    nsub = ct // 128            # 128-token subtiles per chunk
    nchunk = S // ct            # chunks per batch
    npe = NJ - ndmat            # d-tiles transposed on PE per subtile

    nc = bacc.Bacc("TRN2", target_bir_lowering=False, debug=False)

    tokb = nc.declare_dram_parameter("tokb", [B_LOC, S, D], bf16,
                                     isOutput=False)
    if ndmat:
        tokt = nc.declare_dram_parameter(
            "tokt", [B_LOC, TS, 128, ndmat, 128], bf16, isOutput=False)
    qkt = nc.declare_dram_parameter("qkt", [128, NJ, H], bf16, isOutput=False)
    if biased:
        sbrow = nc.declare_dram_parameter("sbrow", [1, H], bf16,
                                          isOutput=False)
    if masked:
        maskb = nc.declare_dram_parameter("maskb", [128, B_LOC * TS], f32,
                                          isOutput=False)
    wvt = nc.declare_dram_parameter("wvt", [NJ, 128, D], bf16, isOutput=False)
    wot = nc.declare_dram_parameter("wot", [NJ, 128, D], bf16, isOutput=False)
    bvec = nc.declare_dram_parameter("bvec", [B_LOC, 4, D], f32,
                                     isOutput=False)
    out = nc.declare_dram_parameter("out", [B_LOC, D], f32, isOutput=True)
    if dbg:
        dtt = nc.declare_dram_parameter("dtt", [128, NJ, 128], f32,
                                        isOutput=True)
        dpt = nc.declare_dram_parameter("dpt", [TS, 128, H], f32,
                                        isOutput=True)
        dtp = nc.declare_dram_parameter("dtp", [128, NJ, H, B_LOC], f32,
                                        isOutput=True)
        dzi = nc.declare_dram_parameter("dzi", [B_LOC, H, 1], f32,
                                        isOutput=True)
        dpo = nc.declare_dram_parameter("dpo", [B_LOC, D], f32,
                                        isOutput=True)
        dx = nc.declare_dram_parameter("dx", [B_LOC, D], f32, isOutput=True)

    with tile.TileContext(nc) as tc:
        with tc.tile_pool(name="singles", bufs=1) as singles:
            ident = singles.tile([128, 128], bf16)
            make_identity(nc, ident)
            identf = singles.tile([H, H], f32)
            make_identity(nc, identf)
            ones_col = singles.tile([128, 1], bf16)
            nc.vector.memset(ones_col, 1.0)
            if biased:
                ones_row = singles.tile([1, 128], bf16)
                nc.vector.memset(ones_row, 1.0)
                sb_sb = singles.tile([1, H], bf16)
                nc.sync.dma_start(out=sb_sb, in_=sbrow.ap())
            zbias = singles.tile([128, 1], f32)
            nc.vector.memset(zbias, 0.0)
            eps_sb = singles.tile([B_LOC, 1], f32)
            nc.vector.memset(eps_sb, EPS)

            qkt_sb = singles.tile([128, NJ, H], bf16)
            nc.sync.dma_start(out=qkt_sb, in_=qkt.ap())
            if masked:
                maskb_sb = singles.tile([128, B_LOC * TS], f32)
                nc.sync.dma_start(out=maskb_sb, in_=maskb.ap())
            bvec_sb = singles.tile([B_LOC, 4, D], f32)
            nc.sync.dma_start(out=bvec_sb, in_=bvec.ap())

            wvt_sb = singles.tile([128, NJ, D], bf16)
            wot_sb = singles.tile([128, NJ, D], bf16)

            tpT_sb = singles.tile([128, NJ, H, B_LOC], bf16)

            for rep in range(reps):
                # weights ride the gpsimd SWDGE ring: separate DGE from the
                # token stream's HWDGE rings, first consumed by the epilogue
                for j in range(NJ):
                    nc.gpsimd.dma_start(out=wvt_sb[:, j, :], in_=wvt.ap()[j])
                for j in range(NJ):
                    nc.gpsimd.dma_start(out=wot_sb[:, j, :], in_=wot.ap()[j])

                with (
                    tc.tile_pool(name="chunks", bufs=chunk_bufs) as chunks,
                    tc.tile_pool(name="ttds", bufs=chunk_bufs) as ttds,
                    tc.tile_pool(name="tts", bufs=tt_bufs) as tts,
                    tc.tile_pool(name="smalls", bufs=3) as smalls,
                    tc.tile_pool(name="ps_tr", bufs=2, space="PSUM") as ps_tr,
                    tc.tile_pool(name="ps_sc", bufs=2, space="PSUM") as ps_sc,
                    tc.tile_pool(name="ps_tp", bufs=1, space="PSUM") as ps_tp,
                    tc.tile_pool(name="ps_z", bufs=1, space="PSUM") as ps_z,
                ):
                    for b in range(B_LOC):
                        # pooled [H, D] in 3 PSUM banks (one long-lived
                        # accumulation region per bank, baseline-style) and
                        # the softmax normalizer Z [H, 1] in a fourth
                        psum_tp = ps_tp.tile([H, D], f32, tag="tp")
                        psum_zb = ps_z.tile([H, 1], f32, tag="zb")

                        def emit_pool(carry, first, last):
                            tokc_p, sub_p, pt_p, _i = carry
                            for k in range(3):
                                nc.tensor.matmul(
                                    psum_tp[:, k * 512:(k + 1) * 512],
                                    pt_p,
                                    tokc_p[:, sub_p, k * 512:(k + 1) * 512],
                                    start=first, stop=last)
                            nc.tensor.matmul(psum_zb, pt_p, ones_col,
                                             start=first, stop=last)

                        carry = None
                        for c in range(nchunk):
                            tokc = chunks.tile([128, nsub, D], bf16,
                                               tag="tok")
                            eng = nc.sync if c % 2 == 0 else nc.scalar
                            oeng = nc.scalar if c % 2 == 0 else nc.sync
                            src = tokb.ap()[b].rearrange(
                                "(c s p) d -> c p s d", s=nsub, p=128)
                            eng.dma_start(out=tokc, in_=src[c])
                            if ndmat:
                                ttd = ttds.tile([128, nsub, ndmat, 128],
                                                bf16, tag="ttd")
                                tsrc = tokt.ap()[b].rearrange(
                                    "(c s) p j q -> c p s j q", s=nsub)
                                oeng.dma_start(out=ttd, in_=tsrc[c])
                            if ablate == "dma":
                                continue
                            for sub in range(nsub):
                                i = c * nsub + sub
                                # PE transposes -> tt (bf16 PSUM -> SBUF)
                                tt = tts.tile([128, npe, 128], bf16,
                                              tag="tt")
                                tt_flat = tt.rearrange("p j s -> p (j s)")
                                ngi = (npe + grp - 1) // grp
                                for g in range(ngi):
                                    g0, g1 = g * grp, min(npe, (g + 1) * grp)
                                    ptr = ps_tr.tile([128, grp * 128], bf16,
                                                     tag="tr")
                                    for q in range(g1 - g0):
                                        j = ndmat + g0 + q
                                        nc.tensor.transpose(
                                            ptr[:, q * 128:(q + 1) * 128],
                                            tokc[:, sub,
                                                 j * 128:(j + 1) * 128],
                                            ident)
                                    dst = tt_flat[:, g0 * 128:g1 * 128]
                                    src_ap = ptr[:, 0:(g1 - g0) * 128]
                                    if copies == "dve" or g % 2 == 0:
                                        nc.vector.tensor_copy(out=dst,
                                                              in_=src_ap)
                                    else:
                                        nc.scalar.copy(out=dst, in_=src_ap)

                                if carry is not None and ablate == "none":
                                    emit_pool(carry, first=(carry[3] == 0),
                                              last=False)
                                if ablate == "tr":
                                    carry = (tokc, sub, None, i)
                                    continue

                                # scoresT[s, h]
                                pssT = ps_sc.tile([128, H], f32, tag="sc")
                                for j in range(NJ):
                                    if j < ndmat:
                                        lhsT = ttd[:, sub, j, :]
                                    else:
                                        lhsT = tt[:, j - ndmat, :]
                                    nc.tensor.matmul(
                                        pssT, lhsT, qkt_sb[:, j, :],
                                        start=(j == 0),
                                        stop=(j == NJ - 1 and not biased))
                                if biased:
                                    nc.tensor.matmul(pssT, ones_row, sb_sb,
                                                     start=False, stop=True)
                                pt = smalls.tile([128, H], bf16, tag="pt")
                                bias = (maskb_sb[:, b * TS + i:b * TS + i + 1]
                                        if masked else zbias)
                                nc.scalar.activation(pt, pssT, Exp,
                                                     bias=bias, scale=1.0)
                                if dbg and b == 0:
                                    nc.gpsimd.dma_start(out=dpt.ap()[i],
                                                        in_=pt)
                                    if i == 0:
                                        nc.gpsimd.dma_start(out=dtt.ap(),
                                                            in_=tt)
                                carry = (tokc, sub, pt, i)
                        if carry is not None and ablate == "none":
                            emit_pool(carry, first=(carry[3] == 0), last=True)
                        carry = None

                        if ablate != "none":
                            continue
                        # batch epilogue: normalize by Z and transpose the
                        # pooled [H, D] into tpT [d, h] for the V-projection
                        linv = smalls.tile([H, 1], f32, tag="linv")
                        nc.vector.reciprocal(linv, psum_zb)
                        tp_sb = smalls.tile([H, D], f32, tag="tpsb")
                        nc.vector.tensor_scalar_mul(tp_sb, psum_tp, linv)
                        for j in range(NJ):
                            ptp = ps_sc.tile([128, H], f32, tag="sc")
                            nc.tensor.transpose(
                                ptp, tp_sb[:, j * 128:(j + 1) * 128],
                                identf[:H, :H])
                            nc.vector.tensor_copy(
                                out=tpT_sb[:, j, :, b], in_=ptp)
                        if dbg:
                            nc.gpsimd.dma_start(out=dzi.ap()[b], in_=linv)

                    if ablate == "none" and dbg:
                        nc.gpsimd.dma_start(out=dtp.ap(), in_=tpT_sb)

                if ablate != "none":
                    with tc.tile_pool(name="abl", bufs=1) as abl:
                        xa = abl.tile([B_LOC, D], f32, tag="xa")
                        nc.vector.memset(xa, 0.0)
                        nc.sync.dma_start(out=out.ap(), in_=xa)
                    continue

                # ---- core epilogue: projections + layernorm ----
                with (
                    tc.tile_pool(name="epil", bufs=1) as epil,
                    tc.tile_pool(name="ps_epi", bufs=1, space="PSUM") as ps_epi,
                ):
                    bv2_sb = bvec_sb[:, 0, :]
                    bo2_sb = bvec_sb[:, 1, :]
                    g2_sb = bvec_sb[:, 2, :]
                    be2_sb = bvec_sb[:, 3, :]

                    # V-projection per head; 256-f32 stride keeps each
                    # matmul inside one PSUM bank
                    psum_vp = ps_epi.tile([B_LOC, H, 256], f32, tag="vp")
                    for h in range(H):
                        for j in range(NJ):
                            nc.tensor.matmul(
                                psum_vp[:, h, 0:HD],
                                tpT_sb[:, j, h, :],
                                wvt_sb[:, j, h * HD:(h + 1) * HD],
                                start=(j == 0), stop=(j == NJ - 1),
                            )
                    pooled_sb = epil.tile([B_LOC, H, HD], bf16, tag="pooled")
                    nc.vector.tensor_add(
                        pooled_sb, psum_vp[:, :, 0:HD],
                        bv2_sb.rearrange("p (h e) -> p h e", h=H))
                    pooled_flat = pooled_sb.rearrange("p h e -> p (h e)")
                    if dbg:
                        nc.gpsimd.dma_start(out=dpo.ap(), in_=pooled_flat)

                    # O-projection: transpose pooled, psum_op = pooledT.T@woT
                    poT_sb = epil.tile([128, NJ, B_LOC], bf16, tag="poT")
                    for j in range(NJ):
                        ppo = ps_epi.tile([128, B_LOC], bf16, tag="po")
                        nc.tensor.transpose(
                            ppo, pooled_flat[:, j * 128:(j + 1) * 128],
                            ident[:B_LOC, :B_LOC])
                        nc.vector.tensor_copy(out=poT_sb[:, j, :], in_=ppo)
                    psum_op = ps_epi.tile([B_LOC, D], f32, tag="op")
                    for j in range(NJ):
                        for k in range(3):
                            nc.tensor.matmul(
                                psum_op[:, k * 512:(k + 1) * 512],
                                poT_sb[:, j, :],
                                wot_sb[:, j, k * 512:(k + 1) * 512],
                                start=(j == 0), stop=(j == NJ - 1))
                    x_sb = epil.tile([B_LOC, D], f32, tag="x")
                    nc.vector.tensor_add(x_sb, psum_op, bo2_sb)
                    if dbg:
                        nc.gpsimd.dma_start(out=dx.ap(), in_=x_sb)

                    # LayerNorm
                    x3 = x_sb.rearrange("p (g q) -> p g q", g=3)
                    stats = epil.tile([B_LOC, 3, 6], f32, tag="stats")
                    for g in range(3):
                        nc.vector.bn_stats(out=stats[:, g, :], in_=x3[:, g, :])
                    mv = epil.tile([B_LOC, 2], f32, tag="mv")
                    nc.vector.bn_aggr(out=mv, in_=stats)
                    sd = epil.tile([B_LOC, 1], f32, tag="sd")
                    nc.scalar.activation(sd, mv[:, 1:2], Sqrt,
                                         bias=eps_sb, scale=1.0)
                    rstd = epil.tile([B_LOC, 1], f32, tag="rstd")
                    nc.vector.reciprocal(rstd, sd)
                    xc = epil.tile([B_LOC, D], f32, tag="xc")
                    nc.vector.tensor_scalar_sub(xc, x_sb, mv[:, 0:1])
                    nc.vector.tensor_scalar_mul(xc, xc, rstd)
                    nc.vector.tensor_mul(xc, xc, g2_sb)
                    nc.vector.tensor_add(xc, xc, be2_sb)
                    nc.sync.dma_start(out=out.ap(), in_=xc)

    nc.compile()
    return nc


def _host_prep(tokens, mask, query, wq, wk, wv, bq, bk, bv, wo, bo, gamma,
               beta, trmode="pe0", ct=CT):
    """Fold the tiny projections; all O(D^2) work in float64 for accuracy."""
    import ml_dtypes
    ndmat = int(trmode[2:]) if trmode.startswith("pe") else 0
    scale = 1.0 / np.sqrt(HD)
    q = (np.asarray(query, np.float64) @ np.asarray(wq, np.float64).T
         + np.asarray(bq, np.float64)).reshape(H, HD)
    qk = np.empty((H, D), np.float64)
    sb = np.empty((1, H), np.float64)
    wk64 = np.asarray(wk, np.float64)
    bk64 = np.asarray(bk, np.float64)
    for h in range(H):
        qk[h] = scale * (q[h] @ wk64[h * HD:(h + 1) * HD, :])
        sb[0, h] = scale * (q[h] @ bk64[h * HD:(h + 1) * HD])
    # qkt[p, j, h] = qk[h, 128j + p]
    qkt = np.ascontiguousarray(
        qk.T.reshape(NJ, 128, H).transpose(1, 0, 2)).astype(ml_dtypes.bfloat16)

    wvt = np.ascontiguousarray(
        np.asarray(wv, np.float32).T.reshape(NJ, 128, D)).astype(
            ml_dtypes.bfloat16)
    wot = np.ascontiguousarray(
        np.asarray(wo, np.float32).T.reshape(NJ, 128, D)).astype(
            ml_dtypes.bfloat16)

    bvec = np.ascontiguousarray(np.broadcast_to(
        np.stack([np.asarray(v, np.float32) for v in (bv, bo, gamma, beta)]),
        (B_LOC, 4, D)))

    common = {"qkt": qkt, "wvt": wvt, "wot": wot, "bvec": bvec}

    masked = not bool(np.all(np.asarray(mask)))
    biased = bool(np.abs(sb).max() > 0)
    if biased:
        common["sbrow"] = np.ascontiguousarray(sb).astype(ml_dtypes.bfloat16)
    if masked:
        # maskb[core][p, b*TS + t] = 0 if mask[core*B_LOC+b, t*128+p] else -1e30
        mf = np.asarray(mask).reshape(N_CORES, B_LOC, TS, 128)
        maskb_all = np.where(mf, 0.0, -1e30).astype(np.float32)
        maskb_all = maskb_all.transpose(0, 3, 1, 2).reshape(
            N_CORES, 128, B_LOC * TS)

    tokens = np.asarray(tokens)
    tokens_bf = tokens.astype(ml_dtypes.bfloat16)
    if ndmat:
        # tokt[b, t, p, j, s] = tok[b, t*128 + s, 128*j + p], j < ndmat
        tt_all = np.ascontiguousarray(
            tokens_bf.reshape(B, TS, 128, NJ, 128)[:, :, :, :ndmat, :]
            .transpose(0, 1, 4, 3, 2))
    in_maps = []
    for core in range(N_CORES):
        m = dict(common)
        m["tokb"] = np.ascontiguousarray(
            tokens_bf[core * B_LOC:(core + 1) * B_LOC])
        if ndmat:
            m["tokt"] = np.ascontiguousarray(
                tt_all[core * B_LOC:(core + 1) * B_LOC])
        if masked:
            m["maskb"] = np.ascontiguousarray(maskb_all[core])
        in_maps.append(m)
    return in_maps


TRMODE = "pe0"


def kernel(tokens, mask, query, wq, wk, wv, bq, bk, bv, wo, bo, gamma, beta):
    from concourse.bass_utils import run_bass_kernel_spmd

    masked = not bool(np.all(np.asarray(mask)))
    in_maps = _host_prep(tokens, mask, query, wq, wk, wv, bq, bk, bv,
                         wo, bo, gamma, beta, trmode=TRMODE)
    biased = "sbrow" in in_maps[0]
    key = ("nc", masked, biased, TRMODE)
    if key not in _CACHE:
        _CACHE[key] = _build_nc(masked=masked, biased=biased, trmode=TRMODE)
    nc = _CACHE[key]
    res = run_bass_kernel_spmd(nc, in_maps, list(range(N_CORES)))
    return np.concatenate([res.results[c]["out"] for c in range(N_CORES)],
                          axis=0).astype(np.float32)


# revision 45
# speedup vs baseline: 1.2563x; 1.2563x over previous
"""AttentionPooling Trainium2 Bass kernel (v2).

Full inputs in, full outputs out. Data-parallel over batch across 8 cores
(2 batches per core). Host folds the query/K projections into one small
[D, H] matrix qkt (scores[b,s,h] = tokens[b,s,:] @ qkt); V/O projections
are deferred until after the sequence reduction.

v2 reads tokens from HBM exactly once, in bf16 (24 MiB/core vs 72 in v1):

  per 128-token subtile:
    - 12 PE transposes of the bf16 token tile -> tt[d, s]  (bf16 PSUM),
      copied to SBUF alternately by DVE / scalar engine
    - scoresT[s, h]: 12 accumulating matmuls, lhsT = tt_j, rhs = qkt_j
    - exp on the scalar engine; the key-padding mask rides the per-
      partition activation bias (tokens are partitions here)
    - pooledT[d, h] accumulates in PSUM via lhsT = token tile (stationary),
      rhs = exp(scoresT); the softmax normalizer Z is one extra ones-column
      matmul into a [1, H] PSUM accumulator

  The 1/Z normalization is applied after the V-projection, where Z is a
  per-partition [B_LOC, 1] scalar per head block. Weights are bf16 and
  stream on the gpsimd SWDGE ring, overlapping the token stream.

Optional trmode "peN": N of the 12 d-tiles per subtile come pre-transposed
from HBM (host-prepared layout, contiguous DMA) instead of PE transposes,
trading DMA bytes for PE cycles.
"""

import numpy as np

B, S, D, H = 16, 4096, 1536, 8
HD = D // H                     # 192
N_CORES = 8
B_LOC = B // N_CORES            # 2 batches per core
NJ = D // 128                   # 12 d-tiles
TS = S // 128                   # 32 subtiles per batch
CT = 512                        # tokens per streamed chunk
EPS = 1e-6

_CACHE = {}


def _build_nc(reps=1, ct=CT, chunk_bufs=3, tt_bufs=3, masked=False,
              biased=False, trmode="pe0", copies="mix", ablate="none",
              grp=6, dbg=False, dr=False):
    import concourse.bacc as bacc
    import concourse.tile as tile
    from concourse import mybir
    from concourse.masks import make_identity

    dr = trmode.endswith("f8")
    core = trmode[:-2] if dr else trmode
    ndmat = int(core[2:]) if core.startswith("pe") else 0

    f32 = mybir.dt.float32
    bf16 = mybir.dt.bfloat16
    f8 = mybir.dt.float8e4
    Exp = mybir.ActivationFunctionType.Exp
    Sqrt = mybir.ActivationFunctionType.Sqrt

    nsub = ct // 128            # 128-token subtiles per chunk
    nchunk = S // ct            # chunks per batch
    npe = NJ - ndmat            # d-tiles transposed on PE per subtile
    if dr:
        # fp8 DoubleRow for the scores pass only: d-tile pairs contract 256
        # deep per matmul; plane-major operand layout matches the tiles
        # as-is. qkt is host-scaled by 256 (fp8 subnormal dodge), descaled
        # in the exp activation.
        assert ndmat == 0 and not biased
        DR = mybir.MatmulPerfMode.DoubleRow

    nc = bacc.Bacc("TRN2", target_bir_lowering=False, debug=False)

    tokb = nc.declare_dram_parameter("tokb", [B_LOC, S, D], bf16,
                                     isOutput=False)
    if ndmat:
        tokt = nc.declare_dram_parameter(
            "tokt", [B_LOC, TS, 128, ndmat, 128], bf16, isOutput=False)
    qkt = nc.declare_dram_parameter("qkt", [128, NJ, H], f8 if dr else bf16,
                                    isOutput=False)
    if biased:
        sbrow = nc.declare_dram_parameter("sbrow", [1, H], bf16,
                                          isOutput=False)
    if masked:
        maskb = nc.declare_dram_parameter("maskb", [128, B_LOC * TS], f32,
                                          isOutput=False)
    wvt = nc.declare_dram_parameter("wvt", [NJ, 128, D], bf16, isOutput=False)
    wot = nc.declare_dram_parameter("wot", [NJ, 128, D], bf16, isOutput=False)
    bvec = nc.declare_dram_parameter("bvec", [B_LOC, 4, D], f32,
                                     isOutput=False)
    out = nc.declare_dram_parameter("out", [B_LOC, D], f32, isOutput=True)
    if dbg:
        dtt = nc.declare_dram_parameter("dtt", [128, NJ, 128], f32,
                                        isOutput=True)
        dpt = nc.declare_dram_parameter("dpt", [TS, 128, H], f32,
                                        isOutput=True)
        dtp = nc.declare_dram_parameter("dtp", [128, NJ, H, B_LOC], f32,
                                        isOutput=True)
        dzi = nc.declare_dram_parameter("dzi", [B_LOC, H, 1], f32,
                                        isOutput=True)
        dpo = nc.declare_dram_parameter("dpo", [B_LOC, D], f32,
                                        isOutput=True)
        dx = nc.declare_dram_parameter("dx", [B_LOC, D], f32, isOutput=True)

    with tile.TileContext(nc) as tc:
        with tc.tile_pool(name="singles", bufs=1) as singles:
            ident = singles.tile([128, 128], bf16)
            make_identity(nc, ident)
            identf = singles.tile([H, H], f32)
            make_identity(nc, identf)
            ones_col = singles.tile([128, 1], bf16)
            nc.vector.memset(ones_col, 1.0)
            if biased:
                ones_row = singles.tile([1, 128], bf16)
                nc.vector.memset(ones_row, 1.0)
                sb_sb = singles.tile([1, H], bf16)
                nc.sync.dma_start(out=sb_sb, in_=sbrow.ap())
            zbias = singles.tile([128, 1], f32)
            nc.vector.memset(zbias, 0.0)
            eps_sb = singles.tile([B_LOC, 1], f32)
            nc.vector.memset(eps_sb, EPS)

            qkt_sb = singles.tile([128, NJ, H], f8 if dr else bf16)
            nc.sync.dma_start(out=qkt_sb, in_=qkt.ap())
            if masked:
                maskb_sb = singles.tile([128, B_LOC * TS], f32)
                nc.sync.dma_start(out=maskb_sb, in_=maskb.ap())
            bvec_sb = singles.tile([B_LOC, 4, D], f32)
            nc.sync.dma_start(out=bvec_sb, in_=bvec.ap())

            wvt_sb = singles.tile([128, NJ, D], bf16)
            wot_sb = singles.tile([128, NJ, D], bf16)

            tpT_sb = singles.tile([128, NJ, H, B_LOC], bf16)

            for rep in range(reps):
                # weights ride the gpsimd SWDGE ring: separate DGE from the
                # token stream's HWDGE rings, first consumed by the epilogue
                for j in range(NJ):
                    nc.gpsimd.dma_start(out=wvt_sb[:, j, :], in_=wvt.ap()[j])
                for j in range(NJ):
                    nc.gpsimd.dma_start(out=wot_sb[:, j, :], in_=wot.ap()[j])

                with (
                    tc.tile_pool(name="chunks", bufs=chunk_bufs) as chunks,
                    tc.tile_pool(name="ttds", bufs=chunk_bufs) as ttds,
                    tc.tile_pool(name="tts", bufs=tt_bufs) as tts,
                    tc.tile_pool(name="smalls", bufs=3) as smalls,
                    tc.tile_pool(name="ps_tr", bufs=2, space="PSUM") as ps_tr,
                    tc.tile_pool(name="ps_sc", bufs=2, space="PSUM") as ps_sc,
                    tc.tile_pool(name="ps_tp", bufs=1, space="PSUM") as ps_tp,
                    tc.tile_pool(name="ps_z", bufs=1, space="PSUM") as ps_z,
                ):
                    for b in range(B_LOC):
                        # pooled [H, D] in 3 PSUM banks (one long-lived
                        # accumulation region per bank, baseline-style) and
                        # the softmax normalizer Z [H, 1] in a fourth
                        psum_tp = ps_tp.tile([H, D], f32, tag="tp")
                        psum_zb = ps_z.tile([H, 1], f32, tag="zb")

                        def emit_pool(carry, first, last):
                            tokc_p, sub_p, pt_p, _i = carry
                            for k in range(3):
                                nc.tensor.matmul(
                                    psum_tp[:, k * 512:(k + 1) * 512],
                                    pt_p,
                                    tokc_p[:, sub_p, k * 512:(k + 1) * 512],
                                    start=first, stop=last)
                            nc.tensor.matmul(psum_zb, pt_p, ones_col,
                                             start=first, stop=last)

                        carry = None
                        for c in range(nchunk):
                            tokc = chunks.tile([128, nsub, D], bf16,
                                               tag="tok")
                            eng = nc.sync if c % 2 == 0 else nc.scalar
                            oeng = nc.scalar if c % 2 == 0 else nc.sync
                            src = tokb.ap()[b].rearrange(
                                "(c s p) d -> c p s d", s=nsub, p=128)
                            eng.dma_start(out=tokc, in_=src[c])
                            if ndmat:
                                ttd = ttds.tile([128, nsub, ndmat, 128],
                                                bf16, tag="ttd")
                                tsrc = tokt.ap()[b].rearrange(
                                    "(c s) p j q -> c p s j q", s=nsub)
                                oeng.dma_start(out=ttd, in_=tsrc[c])
                            if ablate == "dma":
                                continue
                            for sub in range(nsub):
                                i = c * nsub + sub
                                # PE transposes -> tt (bf16 PSUM -> SBUF)
                                tt = tts.tile([128, npe, 128],
                                              f8 if dr else bf16, tag="tt")
                                tt_flat = tt.rearrange("p j s -> p (j s)")
                                ngi = (npe + grp - 1) // grp
                                for g in range(ngi):
                                    g0, g1 = g * grp, min(npe, (g + 1) * grp)
                                    ptr = ps_tr.tile([128, grp * 128], bf16,
                                                     tag="tr")
                                    for q in range(g1 - g0):
                                        j = ndmat + g0 + q
                                        nc.tensor.transpose(
                                            ptr[:, q * 128:(q + 1) * 128],
                                            tokc[:, sub,
                                                 j * 128:(j + 1) * 128],
                                            ident)
                                    dst = tt_flat[:, g0 * 128:g1 * 128]
                                    src_ap = ptr[:, 0:(g1 - g0) * 128]
                                    if copies == "dve" or g % 2 == 0:
                                        nc.vector.tensor_copy(out=dst,
                                                              in_=src_ap)
                                    else:
                                        nc.scalar.copy(out=dst, in_=src_ap)

                                if carry is not None and ablate == "none":
                                    emit_pool(carry, first=(carry[3] == 0),
                                              last=False)
                                    carry = None
                                if ablate == "tr":
                                    carry = None
                                    continue

                                # scoresT[s, h]
                                pssT = ps_sc.tile([128, H], f32, tag="sc")
                                if dr:
                                    for jj in range(NJ // 2):
                                        nc.tensor.matmul(
                                            pssT,
                                            tt[:, 2 * jj:2 * jj + 2, :],
                                            qkt_sb[:, 2 * jj:2 * jj + 2, :],
                                            start=(jj == 0),
                                            stop=(jj == NJ // 2 - 1),
                                            perf_mode=DR)
                                else:
                                    for j in range(NJ):
                                        if j < ndmat:
                                            lhsT = ttd[:, sub, j, :]
                                        else:
                                            lhsT = tt[:, j - ndmat, :]
                                        nc.tensor.matmul(
                                            pssT, lhsT, qkt_sb[:, j, :],
                                            start=(j == 0),
                                            stop=(j == NJ - 1 and not biased))
                                    if biased:
                                        nc.tensor.matmul(pssT, ones_row,
                                                         sb_sb,
                                                         start=False,
                                                         stop=True)
                                pt = smalls.tile([128, H], bf16, tag="pt")
                                bias = (maskb_sb[:, b * TS + i:b * TS + i + 1]
                                        if masked else zbias)
                                nc.scalar.activation(pt, pssT, Exp,
                                                     bias=bias,
                                                     scale=(1.0 / 256
                                                            if dr else 1.0))
                                if dbg and b == 0:
                                    nc.gpsimd.dma_start(out=dpt.ap()[i],
                                                        in_=pt)
                                    if i == 0:
                                        nc.gpsimd.dma_start(out=dtt.ap(),
                                                            in_=tt)
                                carry = (tokc, sub, pt, i)
                        if carry is not None and ablate == "none":
                            emit_pool(carry, first=(carry[3] == 0), last=True)
                        carry = None

                        if ablate != "none":
                            continue
                        # batch epilogue: normalize by Z and transpose the
                        # pooled [H, D] into tpT [d, h] for the V-projection
                        linv = smalls.tile([H, 1], f32, tag="linv")
                        nc.vector.reciprocal(linv, psum_zb)
                        tp_sb = smalls.tile([H, D], f32, tag="tpsb")
                        nc.vector.tensor_scalar_mul(tp_sb, psum_tp, linv)
                        for j in range(NJ):
                            ptp = ps_sc.tile([128, H], f32, tag="sc")
                            nc.tensor.transpose(
                                ptp, tp_sb[:, j * 128:(j + 1) * 128],
                                identf[:H, :H])
                            nc.vector.tensor_copy(
                                out=tpT_sb[:, j, :, b], in_=ptp)
                        if dbg:
                            nc.gpsimd.dma_start(out=dzi.ap()[b], in_=linv)

                    if ablate == "none" and dbg:
                        nc.gpsimd.dma_start(out=dtp.ap(), in_=tpT_sb)

                if ablate != "none":
                    with tc.tile_pool(name="abl", bufs=1) as abl:
                        xa = abl.tile([B_LOC, D], f32, tag="xa")
                        nc.vector.memset(xa, 0.0)
                        nc.sync.dma_start(out=out.ap(), in_=xa)
                    continue

                # ---- core epilogue: projections + layernorm ----
                with (
                    tc.tile_pool(name="epil", bufs=1) as epil,
                    tc.tile_pool(name="ps_epi", bufs=1, space="PSUM") as ps_epi,
                ):
                    bv2_sb = bvec_sb[:, 0, :]
                    bo2_sb = bvec_sb[:, 1, :]
                    g2_sb = bvec_sb[:, 2, :]
                    be2_sb = bvec_sb[:, 3, :]

                    # V-projection per head; 256-f32 stride keeps each
                    # matmul inside one PSUM bank
                    psum_vp = ps_epi.tile([B_LOC, H, 256], f32, tag="vp")
                    for h in range(H):
                        for j in range(NJ):
                            nc.tensor.matmul(
                                psum_vp[:, h, 0:HD],
                                tpT_sb[:, j, h, :],
                                wvt_sb[:, j, h * HD:(h + 1) * HD],
                                start=(j == 0), stop=(j == NJ - 1),
                            )
                    pooled_sb = epil.tile([B_LOC, H, HD], bf16, tag="pooled")
                    nc.vector.tensor_add(
                        pooled_sb, psum_vp[:, :, 0:HD],
                        bv2_sb.rearrange("p (h e) -> p h e", h=H))
                    pooled_flat = pooled_sb.rearrange("p h e -> p (h e)")
                    if dbg:
                        nc.gpsimd.dma_start(out=dpo.ap(), in_=pooled_flat)

                    # O-projection: transpose pooled, psum_op = pooledT.T@woT
                    poT_sb = epil.tile([128, NJ, B_LOC], bf16, tag="poT")
                    for j in range(NJ):
                        ppo = ps_epi.tile([128, B_LOC], bf16, tag="po")
                        nc.tensor.transpose(
                            ppo, pooled_flat[:, j * 128:(j + 1) * 128],
                            ident[:B_LOC, :B_LOC])
                        nc.vector.tensor_copy(out=poT_sb[:, j, :], in_=ppo)
                    psum_op = ps_epi.tile([B_LOC, D], f32, tag="op")
                    for j in range(NJ):
                        for k in range(3):
                            nc.tensor.matmul(
                                psum_op[:, k * 512:(k + 1) * 512],
                                poT_sb[:, j, :],
                                wot_sb[:, j, k * 512:(k + 1) * 512],
                                start=(j == 0), stop=(j == NJ - 1))
                    x_sb = epil.tile([B_LOC, D], f32, tag="x")
                    nc.vector.tensor_add(x_sb, psum_op, bo2_sb)
                    if dbg:
                        nc.gpsimd.dma_start(out=dx.ap(), in_=x_sb)

                    # LayerNorm
                    x3 = x_sb.rearrange("p (g q) -> p g q", g=3)
                    stats = epil.tile([B_LOC, 3, 6], f32, tag="stats")
                    for g in range(3):
                        nc.vector.bn_stats(out=stats[:, g, :], in_=x3[:, g, :])
                    mv = epil.tile([B_LOC, 2], f32, tag="mv")
                    nc.vector.bn_aggr(out=mv, in_=stats)
                    sd = epil.tile([B_LOC, 1], f32, tag="sd")
                    nc.scalar.activation(sd, mv[:, 1:2], Sqrt,
                                         bias=eps_sb, scale=1.0)
                    rstd = epil.tile([B_LOC, 1], f32, tag="rstd")
                    nc.vector.reciprocal(rstd, sd)
                    xc = epil.tile([B_LOC, D], f32, tag="xc")
                    nc.vector.tensor_scalar_sub(xc, x_sb, mv[:, 0:1])
                    nc.vector.tensor_scalar_mul(xc, xc, rstd)
                    nc.vector.tensor_mul(xc, xc, g2_sb)
                    nc.vector.tensor_add(xc, xc, be2_sb)
                    nc.sync.dma_start(out=out.ap(), in_=xc)

    nc.compile()
    return nc


def _host_prep(tokens, mask, query, wq, wk, wv, bq, bk, bv, wo, bo, gamma,
               beta, trmode="pe0", ct=CT):
    """Fold the tiny projections; all O(D^2) work in float64 for accuracy."""
    import ml_dtypes
    dr = trmode.endswith("f8")
    core = trmode[:-2] if dr else trmode
    ndmat = int(core[2:]) if core.startswith("pe") else 0
    scale = 1.0 / np.sqrt(HD)
    q = (np.asarray(query, np.float64) @ np.asarray(wq, np.float64).T
         + np.asarray(bq, np.float64)).reshape(H, HD)
    qk = np.empty((H, D), np.float64)
    sb = np.empty((1, H), np.float64)
    wk64 = np.asarray(wk, np.float64)
    bk64 = np.asarray(bk, np.float64)
    for h in range(H):
        qk[h] = scale * (q[h] @ wk64[h * HD:(h + 1) * HD, :])
        sb[0, h] = scale * (q[h] @ bk64[h * HD:(h + 1) * HD])
    # qkt[p, j, h] = qk[h, 128j + p]
    qkt = np.ascontiguousarray(qk.T.reshape(NJ, 128, H).transpose(1, 0, 2))
    if dr:
        # fp8 scores path: scale into fp8's sweet spot; the kernel's exp
        # activation descales by 1/256
        qkt = (qkt * 256.0).astype(ml_dtypes.float8_e4m3)
    else:
        qkt = qkt.astype(ml_dtypes.bfloat16)

    wvt = np.ascontiguousarray(
        np.asarray(wv, np.float32).T.reshape(NJ, 128, D)).astype(
            ml_dtypes.bfloat16)
    wot = np.ascontiguousarray(
        np.asarray(wo, np.float32).T.reshape(NJ, 128, D)).astype(
            ml_dtypes.bfloat16)

    bvec = np.ascontiguousarray(np.broadcast_to(
        np.stack([np.asarray(v, np.float32) for v in (bv, bo, gamma, beta)]),
        (B_LOC, 4, D)))

    common = {"qkt": qkt, "wvt": wvt, "wot": wot, "bvec": bvec}

    masked = not bool(np.all(np.asarray(mask)))
    biased = bool(np.abs(sb).max() > 0)
    if biased:
        common["sbrow"] = np.ascontiguousarray(sb).astype(ml_dtypes.bfloat16)
    if masked:
        # maskb[core][p, b*TS + t] = 0 if mask[core*B_LOC+b, t*128+p] else -1e30
        mf = np.asarray(mask).reshape(N_CORES, B_LOC, TS, 128)
        maskb_all = np.where(mf, 0.0, -1e30).astype(np.float32)
        maskb_all = maskb_all.transpose(0, 3, 1, 2).reshape(
            N_CORES, 128, B_LOC * TS)

    tokens = np.asarray(tokens)
    tokens_bf = tokens.astype(ml_dtypes.bfloat16)
    if ndmat:
        # tokt[b, t, p, j, s] = tok[b, t*128 + s, 128*j + p], j < ndmat
        tt_all = np.ascontiguousarray(
            tokens_bf.reshape(B, TS, 128, NJ, 128)[:, :, :, :ndmat, :]
            .transpose(0, 1, 4, 3, 2))
    in_maps = []
    for core in range(N_CORES):
        m = dict(common)
        m["tokb"] = np.ascontiguousarray(
            tokens_bf[core * B_LOC:(core + 1) * B_LOC])
        if ndmat:
            m["tokt"] = np.ascontiguousarray(
                tt_all[core * B_LOC:(core + 1) * B_LOC])
        if masked:
            m["maskb"] = np.ascontiguousarray(maskb_all[core])
        in_maps.append(m)
    return in_maps


TRMODE = "pe0"


def kernel(tokens, mask, query, wq, wk, wv, bq, bk, bv, wo, bo, gamma, beta):
    from concourse.bass_utils import run_bass_kernel_spmd

    masked = not bool(np.all(np.asarray(mask)))
    in_maps = _host_prep(tokens, mask, query, wq, wk, wv, bq, bk, bv,
                         wo, bo, gamma, beta, trmode=TRMODE)
    biased = "sbrow" in in_maps[0]
    key = ("nc", masked, biased, TRMODE)
    if key not in _CACHE:
        _CACHE[key] = _build_nc(masked=masked, biased=biased, trmode=TRMODE)
    nc = _CACHE[key]
    res = run_bass_kernel_spmd(nc, in_maps, list(range(N_CORES)))
    return np.concatenate([res.results[c]["out"] for c in range(N_CORES)],
                          axis=0).astype(np.float32)
